# revision 3
# baseline (speedup 1.0000x reference)
"""Differential Attention (B=1, S=2048, D=2048, H=16, DH=64) on 8 TRN2 NeuronCores.

Sharding: tensor-parallel over heads — 2 heads per core. Wq/Wk/Wv column-split,
attention fully local per head, Wo row-split; partial outputs summed on host
(the unshard step), which replaces the all-reduce.

v4: score chunks repacked for PE row-tile concurrency.  A score chunk is now
ONE t-tile with both components packed in a single 2-bank PSUM tile
[128, comp, 512]; the two K=64 matmuls (components) land on distinct 64-row
PE tiles (T0/T8) and run concurrently (~222ns/pair vs ~560ns serialized in
v3 — v3's skewed psA freeing made the tile scheduler split the pairs).  One
exp per chunk (N=1024).  Boundary chunks (phase (0,0) scores) use the same
form, emitted inside A1 behind each block's qT/kT copies with a 3/4/4/5
stagger so the psA buf rotation never gates projection matmuls on exps.
PV accumulator pairs (O1,O2) pack into one 1-bank psB tile [128, comp, 256]
(cols 0:129 used; comp-0 t-0 start clears the bank, comp-1 chain rides the
zeroed region).  GN scalar chains batched per phase on [128, 4] tiles.

Per-core pipeline:
  A1: qT/kT projections (in [dh, s] layout) + v (in [s, c] layout) from one
  xt stream (bf16), boundary score chunks interleaved per block.
  Attention phase (b, h): per j-slot [2 chunks | PV comp0 | 2 chunks |
  PV comp1 | GN-j]; chunks belong to the NEXT phase.
  GroupNorm algebra: softmax(s1) - lam*softmax(s2) followed by GroupNorm is
  invariant to per-token positive scaling, so z = O1 - (lam*d1/d2)*O2 with
  eps' = eps*d1^2 in the rsqrt (bit-seed + 2 Newton iters, batched [128,4]).
  gn_weight*(1-lambda_init) folded into Wo rows host-side; gn_bias folded
  into a host-side bias vector.  Block end: PE transposes -> Wo row-chunks.

PSUM (8 banks): psA = [128,2,512]x2 (proj qk pairs / score chunks)
                psB = [128,2,256]x2 (PV pair accumulators / proj v accs)
                psC = [128,512]x2   (transposes, final out)
"""
import math
from contextlib import ExitStack

import numpy as np
import ml_dtypes

import concourse.tile as tile
from concourse import bacc, mybir
from concourse.masks import make_identity
from concourse.bass_utils import run_bass_kernel_spmd

S = 2048          # sequence length
D = 2048          # model dim
H = 16            # heads
DH = 64           # head dim (per component); 2*DH = 128 channels per head
NCORES = 8
HPC = H // NCORES          # heads per core = 2
CPC = HPC * 2 * DH         # channels per core = 256
LAMBDA_INIT = 0.8
EPS = 1e-5

DBLK = 512                 # s-block width
NB = S // DBLK             # 4 s-blocks
KT = D // 128              # 16 k-tiles (contraction for projections)
TT = S // 128              # 16 t-tiles (keys)
F32 = mybir.dt.float32
I32 = mybir.dt.int32
BF16 = mybir.dt.bfloat16
Exp = mybir.ActivationFunctionType.Exp
Op = mybir.AluOpType
NPBF16 = ml_dtypes.bfloat16

# boundary chunk stagger: chunks for phase (0,0) emitted after each A1 block
BND_SCHED = [[0, 1, 2], [3, 4, 5, 6], [7, 8, 9, 10], [11, 12, 13, 14, 15]]


def _build(lam: float):
    nc = bacc.Bacc("TRN2", target_bir_lowering=False, debug=False,
                   num_devices=NCORES)

    xt_d = nc.dram_tensor("xt", [KT, NB, 128, DBLK], BF16, kind="ExternalInput").ap()
    # weights grouped by 4 k-tiles: [g, p, k_in_g * CPC] -> 8KB DMA rows
    wq_d = nc.dram_tensor("wq", [KT // 4, 128, 4 * CPC], BF16, kind="ExternalInput").ap()
    wk_d = nc.dram_tensor("wk", [KT // 4, 128, 4 * CPC], BF16, kind="ExternalInput").ap()
    wv_d = nc.dram_tensor("wv", [KT // 4, 128, 4 * CPC], BF16, kind="ExternalInput").ap()
    wo_d = nc.dram_tensor("wo", [HPC, 128, D], BF16, kind="ExternalInput").ap()
    out_d = nc.dram_tensor("out_p", [S, D], BF16, kind="ExternalOutput").ap()

    with tile.TileContext(nc) as tc, ExitStack() as ctx:
        singles = ctx.enter_context(tc.tile_pool(name="singles", bufs=1))
        xt_pool = ctx.enter_context(tc.tile_pool(name="xt", bufs=6))
        exp_pool = ctx.enter_context(tc.tile_pool(name="exp", bufs=36))
        gn_pool = ctx.enter_context(tc.tile_pool(name="gn", bufs=2))
        ost_pool = ctx.enter_context(tc.tile_pool(name="ost", bufs=3))
        psA = ctx.enter_context(tc.tile_pool(name="psA", bufs=2, space="PSUM"))
        psB = ctx.enter_context(tc.tile_pool(name="psB", bufs=2, space="PSUM"))
        psC = ctx.enter_context(tc.tile_pool(name="psC", bufs=2, space="PSUM"))

        wq_sb = [singles.tile([128, 4, CPC], BF16, tag=f"wq{g}", name=f"wq{g}")
                 for g in range(KT // 4)]
        wk_sb = [singles.tile([128, 4, CPC], BF16, tag=f"wk{g}", name=f"wk{g}")
                 for g in range(KT // 4)]
        wv_sb = [singles.tile([128, 4, CPC], BF16, tag=f"wv{g}", name=f"wv{g}")
                 for g in range(KT // 4)]
        wo_sb = singles.tile([128, HPC, D], BF16, tag="wo")
        # k-group 0 on the fast HWDGE ring (ahead of the xt stream);
        # later groups + wo via SWDGE so they don't block xt tiles.
        nc.sync.dma_start(out=wk_sb[0], in_=wk_d[0])
        nc.sync.dma_start(out=wq_sb[0], in_=wq_d[0])
        for g in range(1, KT // 4):
            nc.gpsimd.dma_start(out=wk_sb[g], in_=wk_d[g])
            nc.gpsimd.dma_start(out=wq_sb[g], in_=wq_d[g])
        for g in range(KT // 4):
            nc.gpsimd.dma_start(out=wv_sb[g], in_=wv_d[g])
        for ct in range(HPC):
            nc.gpsimd.dma_start(out=wo_sb[:, ct, :], in_=wo_d[ct])

        # qT/kT per head: [128 rows = (q1 dims 0:64 | q2 dims 64:128), S]
        qT_sb = [singles.tile([128, S], BF16, tag=f"qT{h}", name=f"qT{h}")
                 for h in range(HPC)]
        kT_sb = [singles.tile([128, S], BF16, tag=f"kT{h}", name=f"kT{h}")
                 for h in range(HPC)]
        # v per t-tile: [128 t, 260]: h0 v 0:128, one 128, pad, h1 v 130:258, one 258
        v_sb = singles.tile([128, TT, 260], BF16, tag="v")
        nc.vector.memset(v_sb[:, :, 128:129], 1.0)
        nc.vector.memset(v_sb[:, :, 258:259], 1.0)

        ident = singles.tile([128, 128], BF16, tag="ident")
        make_identity(nc, ident)
        magic = singles.tile([128, 4], I32, tag="magic")
        nc.vector.memset(magic, 0x5F3759DF)
        one_i = singles.tile([128, 1], I32, tag="one_i")
        nc.vector.memset(one_i, 1)

        # ---- Score chunk: one t-tile, both components, one exp ----
        def emit_score_chunk(b, h, t, tiles):
            sblk = slice(b * DBLK, (b + 1) * DBLK)
            tsl = slice(t * 128, (t + 1) * 128)
            sc = psA.tile([128, 2, DBLK], F32, tag="A", name="sc")
            nc.tensor.matmul(sc[:, 0, :], kT_sb[h][0:64, tsl],
                             qT_sb[h][0:64, sblk], start=True, stop=True)
            nc.tensor.matmul(sc[:, 1, :], kT_sb[h][64:128, tsl],
                             qT_sb[h][64:128, sblk], start=True, stop=True)
            e = exp_pool.tile([128, 2, DBLK], BF16, tag="exp")
            nc.scalar.activation(e, sc, Exp)
            tiles[t] = e

        exp_cur = [[None] * TT for _ in range(HPC)]

        # ---- Stage A1: projections (q,k,v in one xt stream) ----
        for b in range(NB):
            sblk = slice(b * DBLK, (b + 1) * DBLK)
            pq = psA.tile([128, 2, DBLK], F32, tag="A", name="pq")
            pk = psA.tile([128, 2, DBLK], F32, tag="A", name="pk")
            pv = [psB.tile([128, 2, CPC], F32, tag="B", name=f"pv{jj}")
                  for jj in range(2)]
            for k in range(KT):
                xt_t = xt_pool.tile([128, DBLK], BF16, tag="xt")
                nc.sync.dma_start(out=xt_t, in_=xt_d[k, b])
                st, sp = (k == 0), (k == KT - 1)
                g, ki = k // 4, k % 4
                for h in range(HPC):
                    nc.tensor.matmul(
                        pq[:, h, :],
                        wq_sb[g][:, ki, h * 128:(h + 1) * 128],
                        xt_t, start=st, stop=sp)
                    nc.tensor.matmul(
                        pk[:, h, :],
                        wk_sb[g][:, ki, h * 128:(h + 1) * 128],
                        xt_t, start=st, stop=sp)
                for j in range(4):
                    nc.tensor.matmul(pv[j // 2][:, j % 2, :],
                                     xt_t[:, j * 128:(j + 1) * 128],
                                     wv_sb[g][:, ki, :],
                                     start=(st and j % 2 == 0), stop=sp)
            for h in range(HPC):
                nc.vector.tensor_copy(qT_sb[h][:, sblk], pq[:, h, :])
                nc.vector.tensor_copy(kT_sb[h][:, sblk], pk[:, h, :])
            for j in range(4):
                t_idx = b * 4 + j
                nc.vector.tensor_copy(v_sb[:, t_idx, 0:128],
                                      pv[j // 2][:, j % 2, 0:128])
                nc.vector.tensor_copy(v_sb[:, t_idx, 130:258],
                                      pv[j // 2][:, j % 2, 128:256])
            for t in BND_SCHED[b]:
                emit_score_chunk(0, 0, t, exp_cur[0])

        # ---- Attention ----
        def emit_pv_chain(h, j, comp, tiles, opair):
            jsl = slice(j * 128, (j + 1) * 128)
            vsl = slice(h * 130, h * 130 + 129)
            for t in range(TT):
                nc.tensor.matmul(opair[:, comp, 0:129],
                                 tiles[t][:, comp, jsl],
                                 v_sb[:, t, vsl],
                                 start=(comp == 0 and t == 0),
                                 stop=(t == TT - 1))

        def emit_gn_j(j, opair, dd_all, rn_all, z_all, mv_all):
            """Per-j GN front half: denominators, z combine, bn stats."""
            nc.vector.tensor_copy(dd_all[:, j, 0:1], opair[:, 0, 128:129])
            nc.vector.tensor_copy(dd_all[:, j, 1:2], opair[:, 1, 128:129])
            rec = gn_pool.tile([128, 1], F32, tag="rec")
            nc.vector.reciprocal(rec, dd_all[:, j, 1:2])
            nc.vector.tensor_scalar(
                out=rn_all[:, j:j + 1], in0=rec, scalar1=dd_all[:, j, 0:1],
                scalar2=-lam, op0=Op.mult, op1=Op.mult)
            nc.vector.tensor_copy(z_all[:, j, :], opair[:, 0, 0:128])
            nc.vector.scalar_tensor_tensor(
                out=z_all[:, j, :], in0=opair[:, 1, 0:128],
                scalar=rn_all[:, j:j + 1], in1=z_all[:, j, :],
                op0=Op.mult, op1=Op.add)
            stats = gn_pool.tile([128, 6], F32, tag="stats")
            nc.vector.bn_stats(out=stats, in_=z_all[:, j, :])
            nc.vector.bn_aggr(out=mv_all[:, j, :], in_=stats)

        def emit_gn_finish(h, dd_all, mv_all, z_all, xhs):
            """Batched rsqrt on [128,4] + per-j xh normalize."""
            ww = gn_pool.tile([128, 4], F32, tag="ww")
            nc.vector.tensor_tensor(out=ww, in0=dd_all[:, :, 0],
                                    in1=dd_all[:, :, 0], op=Op.mult)
            nc.vector.tensor_scalar(out=ww, in0=ww, scalar1=EPS,
                                    scalar2=None, op0=Op.mult)
            nc.vector.tensor_tensor(out=ww, in0=mv_all[:, :, 1], in1=ww,
                                    op=Op.add)
            sh = gn_pool.tile([128, 4], I32, tag="sh")
            nc.vector.tensor_scalar(
                out=sh, in0=ww.bitcast(I32), scalar1=one_i,
                scalar2=None, op0=Op.arith_shift_right)
            yy = gn_pool.tile([128, 4], F32, tag="yy")
            nc.vector.tensor_tensor(
                out=yy.bitcast(I32), in0=magic, in1=sh, op=Op.subtract)
            for _ in range(2):
                y2 = gn_pool.tile([128, 4], F32, tag="y2")
                nc.vector.tensor_tensor(out=y2, in0=yy, in1=yy, op=Op.mult)
                nc.vector.tensor_tensor(out=y2, in0=y2, in1=ww, op=Op.mult)
                nc.vector.tensor_scalar(
                    out=y2, in0=y2, scalar1=-0.5, scalar2=1.5,
                    op0=Op.mult, op1=Op.add)
                nyy = gn_pool.tile([128, 4], F32, tag="yy")
                nc.vector.tensor_tensor(out=nyy, in0=yy, in1=y2, op=Op.mult)
                yy = nyy
            for j in range(4):
                xh = gn_pool.tile([128, 128], BF16, tag="xh", bufs=10)
                nc.vector.tensor_scalar(
                    out=xh, in0=z_all[:, j, :], scalar1=mv_all[:, j, 0:1],
                    scalar2=yy[:, j:j + 1], op0=Op.subtract, op1=Op.mult)
                xhs[(h, j)] = xh

        def emit_tr(xh):
            trp = psC.tile([128, DBLK], F32, tag="C", name="trp")
            nc.tensor.transpose(trp.bitcast(BF16)[:, 0:128], xh, ident)
            tr = gn_pool.tile([128, 128], BF16, tag="tr", bufs=10)
            nc.vector.tensor_copy(tr, trp.bitcast(BF16)[:, 0:128])
            return tr

        def emit_final(b, j, trs_j):
            srow = (b * 4 + j) * 128
            for n in range(4):
                po = psC.tile([128, DBLK], F32, tag="C", name="po")
                dsl = slice(n * DBLK, (n + 1) * DBLK)
                for ct in range(HPC):
                    nc.tensor.matmul(po, trs_j[ct], wo_sb[:, ct, dsl],
                                     start=(ct == 0), stop=(ct == HPC - 1))
                ostage = ost_pool.tile([128, DBLK], BF16, tag="ost")
                nc.vector.tensor_copy(ostage, po)
                nc.sync.dma_start(out=out_d[srow:srow + 128, dsl], in_=ostage)

        phases = [(b, h) for b in range(NB) for h in range(HPC)]
        xhs = {}
        for pi, (b, h) in enumerate(phases):
            nxt = phases[pi + 1] if pi + 1 < len(phases) else None
            exp_tiles = exp_cur[h]
            if nxt is not None:
                exp_cur[nxt[1]] = [None] * TT
            dd_all = gn_pool.tile([128, 4, 2], F32, tag="dd")
            rn_all = gn_pool.tile([128, 4], F32, tag="rn")
            z_all = gn_pool.tile([128, 4, 128], F32, tag="z")
            mv_all = gn_pool.tile([128, 4, 2], F32, tag="mv")
            for j in range(4):
                if nxt is not None:
                    emit_score_chunk(nxt[0], nxt[1], 4 * j + 0, exp_cur[nxt[1]])
                    emit_score_chunk(nxt[0], nxt[1], 4 * j + 1, exp_cur[nxt[1]])
                opair = psB.tile([128, 2, CPC], F32, tag="B", name="opair")
                emit_pv_chain(h, j, 0, exp_tiles, opair)
                if nxt is not None:
                    emit_score_chunk(nxt[0], nxt[1], 4 * j + 2, exp_cur[nxt[1]])
                    emit_score_chunk(nxt[0], nxt[1], 4 * j + 3, exp_cur[nxt[1]])
                emit_pv_chain(h, j, 1, exp_tiles, opair)
                emit_gn_j(j, opair, dd_all, rn_all, z_all, mv_all)
            emit_gn_finish(h, dd_all, mv_all, z_all, xhs)
            if h == HPC - 1:
                for j in range(4):
                    trs_j = [emit_tr(xhs[(ct, j)]) for ct in range(HPC)]
                    emit_final(b, j, trs_j)

    nc.compile()
    return nc


def prepare(x, Wq, Wk, Wv, Wo, lambda_q1, lambda_k1, lambda_q2, lambda_k2,
            gn_weight, gn_bias):
    """Host-side sharding/preprocessing. Returns (lam, in_maps, bias_vec)."""
    x = np.asarray(x, dtype=np.float32)
    Wq = np.asarray(Wq, dtype=np.float32)
    Wk = np.asarray(Wk, dtype=np.float32)
    Wv = np.asarray(Wv, dtype=np.float32)
    Wo = np.asarray(Wo, dtype=np.float32)
    gw = np.asarray(gn_weight, dtype=np.float32)
    gb = np.asarray(gn_bias, dtype=np.float32)

    lam = float(np.exp(np.sum(np.asarray(lambda_q1, np.float64)
                              * np.asarray(lambda_k1, np.float64)))
                - np.exp(np.sum(np.asarray(lambda_q2, np.float64)
                                * np.asarray(lambda_k2, np.float64)))
                + LAMBDA_INIT)

    xT = np.ascontiguousarray(
        x.reshape(S, D).T.reshape(KT, 128, NB, DBLK).transpose(0, 2, 1, 3)
    ).astype(NPBF16)
    scale = 1.0 / math.sqrt(DH)

    in_maps = []
    for c in range(NCORES):
        sl = slice(c * CPC, (c + 1) * CPC)
        def _grp(w):
            return np.ascontiguousarray(
                w.reshape(KT // 4, 4, 128, CPC).transpose(0, 2, 1, 3)
                .reshape(KT // 4, 128, 4 * CPC)).astype(NPBF16)
        wq_c = _grp(Wq[:, sl] * scale)
        wk_c = _grp(Wk[:, sl])
        wv_c = _grp(Wv[:, sl])
        wo_c = np.ascontiguousarray(
            Wo[sl, :] * ((1.0 - LAMBDA_INIT) * gw[sl])[:, None]
        ).reshape(HPC, 128, D).astype(NPBF16)
        in_maps.append({"xt": xT, "wq": wq_c, "wk": wk_c, "wv": wv_c,
                        "wo": wo_c})

    bias_vec = ((1.0 - LAMBDA_INIT) * gb.astype(np.float64)) @ Wo.astype(np.float64)
    return lam, in_maps, bias_vec


def kernel(x, Wq, Wk, Wv, Wo, lambda_q1, lambda_k1, lambda_q2, lambda_k2,
           gn_weight, gn_bias):
    lam, in_maps, bias_vec = prepare(
        x, Wq, Wk, Wv, Wo, lambda_q1, lambda_k1, lambda_q2, lambda_k2,
        gn_weight, gn_bias)
    nc = _build(lam)
    res = run_bass_kernel_spmd(nc, in_maps, list(range(NCORES)))
    acc = np.zeros((S, D), dtype=np.float64)
    for c in range(NCORES):
        acc += np.asarray(res.results[c]["out_p"], dtype=np.float64)
    acc += bias_vec[None, :]
    return acc.astype(np.float32).reshape(1, S, D)


# revision 8
# speedup vs baseline: 1.1568x; 1.1568x over previous
"""Differential Attention (B=1, S=2048, D=2048, H=16, DH=64) on 8 TRN2 NeuronCores.

Sharding: tensor-parallel over heads — 2 heads per core. Wq/Wk/Wv column-split,
attention fully local per head, Wo row-split; partial outputs summed on host
(the unshard step), which replaces the all-reduce.

v4: score chunks repacked for PE row-tile concurrency.  A score chunk is now
ONE t-tile with both components packed in a single 2-bank PSUM tile
[128, comp, 512]; the two K=64 matmuls (components) land on distinct 64-row
PE tiles (T0/T8) and run concurrently (~222ns/pair vs ~560ns serialized in
v3 — v3's skewed psA freeing made the tile scheduler split the pairs).  One
exp per chunk (N=1024).  Boundary chunks (phase (0,0) scores) use the same
form, emitted inside A1 behind each block's qT/kT copies with a 3/4/4/5
stagger so the psA buf rotation never gates projection matmuls on exps.
PV accumulator pairs (O1,O2) pack into one 1-bank psB tile [128, comp, 256]
(cols 0:129 used; comp-0 t-0 start clears the bank, comp-1 chain rides the
zeroed region).  GN scalar chains batched per phase on [128, 4] tiles.

Per-core pipeline:
  A1: qT/kT projections (in [dh, s] layout) + v (in [s, c] layout) from one
  xt stream (bf16), boundary score chunks interleaved per block.
  Attention phase (b, h): per j-slot [2 chunks | PV comp0 | 2 chunks |
  PV comp1 | GN-j]; chunks belong to the NEXT phase.
  GroupNorm algebra: softmax(s1) - lam*softmax(s2) followed by GroupNorm is
  invariant to per-token positive scaling, so z = O1 - (lam*d1/d2)*O2 with
  eps' = eps*d1^2 in the rsqrt (bit-seed + 2 Newton iters, batched [128,4]).
  gn_weight*(1-lambda_init) folded into Wo rows host-side; gn_bias folded
  into a host-side bias vector.  Block end: PE transposes -> Wo row-chunks.

PSUM (8 banks): psA = [128,2,512]x2 (proj qk pairs / score chunks)
                psB = [128,2,256]x2 (PV pair accumulators / proj v accs)
                psC = [128,512]x2   (transposes, final out)
"""
import math
from contextlib import ExitStack

import numpy as np
import ml_dtypes

import concourse.tile as tile
from concourse import bacc, mybir
from concourse.masks import make_identity
from concourse.bass_utils import run_bass_kernel_spmd

S = 2048          # sequence length
D = 2048          # model dim
H = 16            # heads
DH = 64           # head dim (per component); 2*DH = 128 channels per head
NCORES = 8
HPC = H // NCORES          # heads per core = 2
CPC = HPC * 2 * DH         # channels per core = 256
LAMBDA_INIT = 0.8
EPS = 1e-5

DBLK = 512                 # s-block width
NB = S // DBLK             # 4 s-blocks
KT = D // 128              # 16 k-tiles (contraction for projections)
TT = S // 128              # 16 t-tiles (keys)
F32 = mybir.dt.float32
I32 = mybir.dt.int32
BF16 = mybir.dt.bfloat16
Exp = mybir.ActivationFunctionType.Exp
Op = mybir.AluOpType
NPBF16 = ml_dtypes.bfloat16

# boundary chunk stagger: chunks for phase (0,0) emitted after each A1 block
BND_SCHED = [[0, 1, 2], [3, 4, 5, 6], [7, 8, 9, 10], [11, 12, 13, 14, 15]]


def _build(lam: float):
    nc = bacc.Bacc("TRN2", target_bir_lowering=False, debug=False,
                   num_devices=NCORES)

    xt_d = nc.dram_tensor("xt", [KT, NB, 128, DBLK], BF16, kind="ExternalInput").ap()
    # weights grouped by 4 k-tiles: [g, p, k_in_g * CPC] -> 8KB DMA rows
    wq_d = nc.dram_tensor("wq", [KT // 4, 128, 4 * CPC], BF16, kind="ExternalInput").ap()
    wk_d = nc.dram_tensor("wk", [KT // 4, 128, 4 * CPC], BF16, kind="ExternalInput").ap()
    wv_d = nc.dram_tensor("wv", [KT // 4, 128, 4 * CPC], BF16, kind="ExternalInput").ap()
    wo_d = nc.dram_tensor("wo", [HPC, 128, D], BF16, kind="ExternalInput").ap()
    out_d = nc.dram_tensor("out_p", [S, D], BF16, kind="ExternalOutput").ap()

    with tile.TileContext(nc) as tc, ExitStack() as ctx:
        singles = ctx.enter_context(tc.tile_pool(name="singles", bufs=1))
        xt_pool = ctx.enter_context(tc.tile_pool(name="xt", bufs=6))
        exp_pool = ctx.enter_context(tc.tile_pool(name="exp", bufs=36))
        gn_pool = ctx.enter_context(tc.tile_pool(name="gn", bufs=2))
        ost_pool = ctx.enter_context(tc.tile_pool(name="ost", bufs=3))
        psA = ctx.enter_context(tc.tile_pool(name="psA", bufs=2, space="PSUM"))
        psB = ctx.enter_context(tc.tile_pool(name="psB", bufs=2, space="PSUM"))
        psC = ctx.enter_context(tc.tile_pool(name="psC", bufs=2, space="PSUM"))

        wq_sb = [singles.tile([128, 4, CPC], BF16, tag=f"wq{g}", name=f"wq{g}")
                 for g in range(KT // 4)]
        wk_sb = [singles.tile([128, 4, CPC], BF16, tag=f"wk{g}", name=f"wk{g}")
                 for g in range(KT // 4)]
        wv_sb = [singles.tile([128, 4, CPC], BF16, tag=f"wv{g}", name=f"wv{g}")
                 for g in range(KT // 4)]
        wo_sb = singles.tile([128, HPC, D], BF16, tag="wo")
        # k-group 0 on the fast HWDGE ring (ahead of the xt stream);
        # later groups + wo via SWDGE so they don't block xt tiles.
        nc.sync.dma_start(out=wk_sb[0], in_=wk_d[0])
        nc.sync.dma_start(out=wq_sb[0], in_=wq_d[0])
        for g in range(1, KT // 4):
            nc.gpsimd.dma_start(out=wk_sb[g], in_=wk_d[g])
            nc.gpsimd.dma_start(out=wq_sb[g], in_=wq_d[g])
        for g in range(KT // 4):
            nc.gpsimd.dma_start(out=wv_sb[g], in_=wv_d[g])
        for ct in range(HPC):
            nc.gpsimd.dma_start(out=wo_sb[:, ct, :], in_=wo_d[ct])

        # qT/kT per head: [128 rows = (q1 dims 0:64 | q2 dims 64:128), S]
        qT_sb = [singles.tile([128, S], BF16, tag=f"qT{h}", name=f"qT{h}")
                 for h in range(HPC)]
        kT_sb = [singles.tile([128, S], BF16, tag=f"kT{h}", name=f"kT{h}")
                 for h in range(HPC)]
        # v per t-tile: [128 t, 260]: h0 v 0:128, one 128, pad, h1 v 130:258, one 258
        v_sb = singles.tile([128, TT, 260], BF16, tag="v")
        nc.vector.memset(v_sb[:, :, 128:129], 1.0)
        nc.vector.memset(v_sb[:, :, 258:259], 1.0)

        ident = singles.tile([128, 128], BF16, tag="ident")
        make_identity(nc, ident)
        magic = singles.tile([128, 4], I32, tag="magic")
        nc.vector.memset(magic, 0x5F3759DF)
        one_i = singles.tile([128, 1], I32, tag="one_i")
        nc.vector.memset(one_i, 1)

        # ---- Score chunk: one t-tile, both components, one exp ----
        def emit_score_chunk(b, h, t, tiles):
            sblk = slice(b * DBLK, (b + 1) * DBLK)
            tsl = slice(t * 128, (t + 1) * 128)
            sc = psA.tile([128, 2, DBLK], F32, tag="A", name="sc")
            nc.tensor.matmul(sc[:, 0, :], kT_sb[h][0:64, tsl],
                             qT_sb[h][0:64, sblk], start=True, stop=True)
            nc.tensor.matmul(sc[:, 1, :], kT_sb[h][64:128, tsl],
                             qT_sb[h][64:128, sblk], start=True, stop=True)
            e = exp_pool.tile([128, 2, DBLK], BF16, tag="exp")
            nc.scalar.activation(e, sc, Exp)
            tiles[t] = e

        exp_cur = [[None] * TT for _ in range(HPC)]

        # ---- Stage A1: projections (q,k,v in one xt stream) ----
        for b in range(NB):
            sblk = slice(b * DBLK, (b + 1) * DBLK)
            pq = psA.tile([128, 2, DBLK], F32, tag="A", name="pq")
            pk = psA.tile([128, 2, DBLK], F32, tag="A", name="pk")
            pv = [psB.tile([128, 2, CPC], F32, tag="B", name=f"pv{jj}")
                  for jj in range(2)]
            for k in range(KT):
                xt_t = xt_pool.tile([128, DBLK], BF16, tag="xt")
                nc.sync.dma_start(out=xt_t, in_=xt_d[k, b])
                st, sp = (k == 0), (k == KT - 1)
                g, ki = k // 4, k % 4
                for h in range(HPC):
                    nc.tensor.matmul(
                        pq[:, h, :],
                        wq_sb[g][:, ki, h * 128:(h + 1) * 128],
                        xt_t, start=st, stop=sp)
                    nc.tensor.matmul(
                        pk[:, h, :],
                        wk_sb[g][:, ki, h * 128:(h + 1) * 128],
                        xt_t, start=st, stop=sp)
                for j in range(4):
                    nc.tensor.matmul(pv[j // 2][:, j % 2, :],
                                     xt_t[:, j * 128:(j + 1) * 128],
                                     wv_sb[g][:, ki, :],
                                     start=(st and j % 2 == 0), stop=sp)
            for h in range(HPC):
                nc.vector.tensor_copy(qT_sb[h][:, sblk], pq[:, h, :])
                nc.vector.tensor_copy(kT_sb[h][:, sblk], pk[:, h, :])
            for j in range(4):
                t_idx = b * 4 + j
                nc.vector.tensor_copy(v_sb[:, t_idx, 0:128],
                                      pv[j // 2][:, j % 2, 0:128])
                nc.vector.tensor_copy(v_sb[:, t_idx, 130:258],
                                      pv[j // 2][:, j % 2, 128:256])
            for t in BND_SCHED[b]:
                emit_score_chunk(0, 0, t, exp_cur[0])

        # ---- Attention ----
        def emit_gn_j(j, opair, dd_all, rn_all, z_all, mv_all):
            """Per-j GN front half: denominators, z combine, bn stats."""
            nc.vector.tensor_copy(dd_all[:, j, 0:1], opair[:, 0, 128:129])
            rec = gn_pool.tile([128, 1], F32, tag="rec")
            nc.vector.reciprocal(rec, opair[:, 1, 128:129])
            nc.vector.tensor_scalar(
                out=rn_all[:, j:j + 1], in0=rec, scalar1=dd_all[:, j, 0:1],
                scalar2=-lam, op0=Op.mult, op1=Op.mult)
            nc.vector.tensor_copy(z_all[:, j, :], opair[:, 0, 0:128])
            nc.vector.scalar_tensor_tensor(
                out=z_all[:, j, :], in0=opair[:, 1, 0:128],
                scalar=rn_all[:, j:j + 1], in1=z_all[:, j, :],
                op0=Op.mult, op1=Op.add)
            stats = gn_pool.tile([128, 6], F32, tag="stats")
            nc.vector.bn_stats(out=stats, in_=z_all[:, j, :])
            nc.vector.bn_aggr(out=mv_all[:, j, :], in_=stats)

        def emit_gn_finish(h, dd_all, mv_all, z_all, xhs):
            """Batched rsqrt on [128,4] + per-j xh normalize."""
            ww = gn_pool.tile([128, 4], F32, tag="ww")
            nc.vector.tensor_tensor(out=ww, in0=dd_all[:, :, 0],
                                    in1=dd_all[:, :, 0], op=Op.mult)
            nc.vector.tensor_scalar(out=ww, in0=ww, scalar1=EPS,
                                    scalar2=None, op0=Op.mult)
            nc.vector.tensor_tensor(out=ww, in0=mv_all[:, :, 1], in1=ww,
                                    op=Op.add)
            sh = gn_pool.tile([128, 4], I32, tag="sh")
            nc.vector.tensor_scalar(
                out=sh, in0=ww.bitcast(I32), scalar1=one_i,
                scalar2=None, op0=Op.arith_shift_right)
            yy = gn_pool.tile([128, 4], F32, tag="yy")
            nc.vector.tensor_tensor(
                out=yy.bitcast(I32), in0=magic, in1=sh, op=Op.subtract)
            for _ in range(2):
                y2 = gn_pool.tile([128, 4], F32, tag="y2")
                nc.vector.tensor_tensor(out=y2, in0=yy, in1=yy, op=Op.mult)
                nc.vector.tensor_tensor(out=y2, in0=y2, in1=ww, op=Op.mult)
                nc.vector.tensor_scalar(
                    out=y2, in0=y2, scalar1=-0.5, scalar2=1.5,
                    op0=Op.mult, op1=Op.add)
                nyy = gn_pool.tile([128, 4], F32, tag="yy")
                nc.vector.tensor_tensor(out=nyy, in0=yy, in1=y2, op=Op.mult)
                yy = nyy
            for j in range(4):
                xh = gn_pool.tile([128, 128], BF16, tag="xh", bufs=10)
                nc.vector.tensor_scalar(
                    out=xh, in0=z_all[:, j, :], scalar1=mv_all[:, j, 0:1],
                    scalar2=yy[:, j:j + 1], op0=Op.subtract, op1=Op.mult)
                xhs[(h, j)] = xh

        def emit_tr(xh):
            trp = psC.tile([128, DBLK], F32, tag="C", name="trp")
            nc.tensor.transpose(trp.bitcast(BF16)[:, 0:128], xh, ident)
            tr = gn_pool.tile([128, 128], BF16, tag="tr", bufs=10)
            nc.vector.tensor_copy(tr, trp.bitcast(BF16)[:, 0:128])
            return tr

        def emit_final(b, j, trs_j):
            srow = (b * 4 + j) * 128
            for n in range(4):
                po = psC.tile([128, DBLK], F32, tag="C", name="po")
                dsl = slice(n * DBLK, (n + 1) * DBLK)
                for ct in range(HPC):
                    nc.tensor.matmul(po, trs_j[ct], wo_sb[:, ct, dsl],
                                     start=(ct == 0), stop=(ct == HPC - 1))
                ostage = ost_pool.tile([128, DBLK], BF16, tag="ost")
                nc.vector.tensor_copy(ostage, po)
                nc.sync.dma_start(out=out_d[srow:srow + 128, dsl], in_=ostage)

        phases = [(b, h) for b in range(NB) for h in range(HPC)]
        xhs = {}
        for pi, (b, h) in enumerate(phases):
            nxt = phases[pi + 1] if pi + 1 < len(phases) else None
            exp_tiles = exp_cur[h]
            if nxt is not None:
                exp_cur[nxt[1]] = [None] * TT
            dd_all = gn_pool.tile([128, 4, 2], F32, tag="dd")
            rn_all = gn_pool.tile([128, 4], F32, tag="rn")
            z_all = gn_pool.tile([128, 4, 128], F32, tag="z")
            mv_all = gn_pool.tile([128, 4, 2], F32, tag="mv")
            def emit_pv_half(j, comp, half, opair):
                jsl = slice(j * 128, (j + 1) * 128)
                vsl = slice(h * 130, h * 130 + 129)
                for t in range(half * 8, half * 8 + 8):
                    nc.tensor.matmul(opair[:, comp, 0:129],
                                     exp_tiles[t][:, comp, jsl],
                                     v_sb[:, t, vsl],
                                     start=(comp == 0 and t == 0),
                                     stop=(t == TT - 1))

            for j in range(4):
                opair = psB.tile([128, 2, CPC], F32, tag="B", name="opair")
                for ci, (comp, half) in enumerate(
                        ((0, 0), (0, 1), (1, 0), (1, 1))):
                    if nxt is not None:
                        emit_score_chunk(nxt[0], nxt[1], 4 * j + ci,
                                         exp_cur[nxt[1]])
                    emit_pv_half(j, comp, half, opair)
                emit_gn_j(j, opair, dd_all, rn_all, z_all, mv_all)
            emit_gn_finish(h, dd_all, mv_all, z_all, xhs)
            if h == HPC - 1:
                for j in range(4):
                    trs_j = [emit_tr(xhs[(ct, j)]) for ct in range(HPC)]
                    emit_final(b, j, trs_j)

    nc.compile()
    return nc


def prepare(x, Wq, Wk, Wv, Wo, lambda_q1, lambda_k1, lambda_q2, lambda_k2,
            gn_weight, gn_bias):
    """Host-side sharding/preprocessing. Returns (lam, in_maps, bias_vec)."""
    x = np.asarray(x, dtype=np.float32)
    Wq = np.asarray(Wq, dtype=np.float32)
    Wk = np.asarray(Wk, dtype=np.float32)
    Wv = np.asarray(Wv, dtype=np.float32)
    Wo = np.asarray(Wo, dtype=np.float32)
    gw = np.asarray(gn_weight, dtype=np.float32)
    gb = np.asarray(gn_bias, dtype=np.float32)

    lam = float(np.exp(np.sum(np.asarray(lambda_q1, np.float64)
                              * np.asarray(lambda_k1, np.float64)))
                - np.exp(np.sum(np.asarray(lambda_q2, np.float64)
                                * np.asarray(lambda_k2, np.float64)))
                + LAMBDA_INIT)

    xT = np.ascontiguousarray(
        x.reshape(S, D).T.reshape(KT, 128, NB, DBLK).transpose(0, 2, 1, 3)
    ).astype(NPBF16)
    scale = 1.0 / math.sqrt(DH)

    in_maps = []
    for c in range(NCORES):
        sl = slice(c * CPC, (c + 1) * CPC)
        def _grp(w):
            return np.ascontiguousarray(
                w.reshape(KT // 4, 4, 128, CPC).transpose(0, 2, 1, 3)
                .reshape(KT // 4, 128, 4 * CPC)).astype(NPBF16)
        wq_c = _grp(Wq[:, sl] * scale)
        wk_c = _grp(Wk[:, sl])
        wv_c = _grp(Wv[:, sl])
        wo_c = np.ascontiguousarray(
            Wo[sl, :] * ((1.0 - LAMBDA_INIT) * gw[sl])[:, None]
        ).reshape(HPC, 128, D).astype(NPBF16)
        in_maps.append({"xt": xT, "wq": wq_c, "wk": wk_c, "wv": wv_c,
                        "wo": wo_c})

    bias_vec = ((1.0 - LAMBDA_INIT) * gb.astype(np.float64)) @ Wo.astype(np.float64)
    return lam, in_maps, bias_vec


def kernel(x, Wq, Wk, Wv, Wo, lambda_q1, lambda_k1, lambda_q2, lambda_k2,
           gn_weight, gn_bias):
    lam, in_maps, bias_vec = prepare(
        x, Wq, Wk, Wv, Wo, lambda_q1, lambda_k1, lambda_q2, lambda_k2,
        gn_weight, gn_bias)
    nc = _build(lam)
    res = run_bass_kernel_spmd(nc, in_maps, list(range(NCORES)))
    acc = np.zeros((S, D), dtype=np.float64)
    for c in range(NCORES):
        acc += np.asarray(res.results[c]["out_p"], dtype=np.float64)
    acc += bias_vec[None, :]
    return acc.astype(np.float32).reshape(1, S, D)


# revision 10
# speedup vs baseline: 1.1602x; 1.0029x over previous
"""Differential Attention (B=1, S=2048, D=2048, H=16, DH=64) on 8 TRN2 NeuronCores.

Sharding: tensor-parallel over heads — 2 heads per core. Wq/Wk/Wv column-split,
attention fully local per head, Wo row-split; partial outputs summed on host
(the unshard step), which replaces the all-reduce.

v4: score chunks repacked for PE row-tile concurrency.  A score chunk is now
ONE t-tile with both components packed in a single 2-bank PSUM tile
[128, comp, 512]; the two K=64 matmuls (components) land on distinct 64-row
PE tiles (T0/T8) and run concurrently (~222ns/pair vs ~560ns serialized in
v3 — v3's skewed psA freeing made the tile scheduler split the pairs).  One
exp per chunk (N=1024).  Boundary chunks (phase (0,0) scores) use the same
form, emitted inside A1 behind each block's qT/kT copies with a 3/4/4/5
stagger so the psA buf rotation never gates projection matmuls on exps.
PV accumulator pairs (O1,O2) pack into one 1-bank psB tile [128, comp, 256]
(cols 0:129 used; comp-0 t-0 start clears the bank, comp-1 chain rides the
zeroed region).  GN scalar chains batched per phase on [128, 4] tiles.

Per-core pipeline:
  A1: qT/kT projections (in [dh, s] layout) + v (in [s, c] layout) from one
  xt stream (bf16), boundary score chunks interleaved per block.
  Attention phase (b, h): per j-slot [2 chunks | PV comp0 | 2 chunks |
  PV comp1 | GN-j]; chunks belong to the NEXT phase.
  GroupNorm algebra: softmax(s1) - lam*softmax(s2) followed by GroupNorm is
  invariant to per-token positive scaling, so z = O1 - (lam*d1/d2)*O2 with
  eps' = eps*d1^2 in the rsqrt (bit-seed + 2 Newton iters, batched [128,4]).
  gn_weight*(1-lambda_init) folded into Wo rows host-side; gn_bias folded
  into a host-side bias vector.  Block end: PE transposes -> Wo row-chunks.

PSUM (8 banks): psA = [128,2,512]x2 (proj qk pairs / score chunks)
                psB = [128,2,256]x2 (PV pair accumulators / proj v accs)
                psC = [128,512]x2   (transposes, final out)
"""
import math
from contextlib import ExitStack

import numpy as np
import ml_dtypes

import concourse.tile as tile
from concourse import bacc, mybir
from concourse.masks import make_identity
from concourse.bass_utils import run_bass_kernel_spmd

S = 2048          # sequence length
D = 2048          # model dim
H = 16            # heads
DH = 64           # head dim (per component); 2*DH = 128 channels per head
NCORES = 8
HPC = H // NCORES          # heads per core = 2
CPC = HPC * 2 * DH         # channels per core = 256
LAMBDA_INIT = 0.8
EPS = 1e-5

DBLK = 512                 # s-block width
NB = S // DBLK             # 4 s-blocks
KT = D // 128              # 16 k-tiles (contraction for projections)
TT = S // 128              # 16 t-tiles (keys)
F32 = mybir.dt.float32
I32 = mybir.dt.int32
BF16 = mybir.dt.bfloat16
Exp = mybir.ActivationFunctionType.Exp
Op = mybir.AluOpType
NPBF16 = ml_dtypes.bfloat16

# boundary chunk stagger: chunks for phase (0,0) emitted after each A1 block
BND_SCHED = [[0, 1, 2], [3, 4, 5, 6], [7, 8, 9, 10], [11, 12, 13, 14, 15]]


def _build(lam: float):
    nc = bacc.Bacc("TRN2", target_bir_lowering=False, debug=False,
                   num_devices=NCORES)

    xt_d = nc.dram_tensor("xt", [KT, NB, 128, DBLK], BF16, kind="ExternalInput").ap()
    # weights grouped by 4 k-tiles: [g, p, k_in_g * CPC] -> 8KB DMA rows
    wq_d = nc.dram_tensor("wq", [KT // 4, 128, 4 * CPC], BF16, kind="ExternalInput").ap()
    wk_d = nc.dram_tensor("wk", [KT // 4, 128, 4 * CPC], BF16, kind="ExternalInput").ap()
    wv_d = nc.dram_tensor("wv", [KT // 4, 128, 4 * CPC], BF16, kind="ExternalInput").ap()
    wo_d = nc.dram_tensor("wo", [HPC, 128, D], BF16, kind="ExternalInput").ap()
    out_d = nc.dram_tensor("out_p", [S, D], BF16, kind="ExternalOutput").ap()

    with tile.TileContext(nc) as tc, ExitStack() as ctx:
        singles = ctx.enter_context(tc.tile_pool(name="singles", bufs=1))
        xt_pool = ctx.enter_context(tc.tile_pool(name="xt", bufs=6))
        exp_pool = ctx.enter_context(tc.tile_pool(name="exp", bufs=36))
        gn_pool = ctx.enter_context(tc.tile_pool(name="gn", bufs=2))
        ost_pool = ctx.enter_context(tc.tile_pool(name="ost", bufs=3))
        psA = ctx.enter_context(tc.tile_pool(name="psA", bufs=3, space="PSUM"))
        psB = ctx.enter_context(tc.tile_pool(name="psB", bufs=2, space="PSUM"))

        wq_sb = [singles.tile([128, 4, CPC], BF16, tag=f"wq{g}", name=f"wq{g}")
                 for g in range(KT // 4)]
        wk_sb = [singles.tile([128, 4, CPC], BF16, tag=f"wk{g}", name=f"wk{g}")
                 for g in range(KT // 4)]
        wv_sb = [singles.tile([128, 4, CPC], BF16, tag=f"wv{g}", name=f"wv{g}")
                 for g in range(KT // 4)]
        wo_sb = singles.tile([128, HPC, D], BF16, tag="wo")
        # k-group 0 on the fast HWDGE ring (ahead of the xt stream);
        # later groups + wo via SWDGE so they don't block xt tiles.
        nc.sync.dma_start(out=wk_sb[0], in_=wk_d[0])
        nc.sync.dma_start(out=wq_sb[0], in_=wq_d[0])
        for g in range(1, KT // 4):
            nc.gpsimd.dma_start(out=wk_sb[g], in_=wk_d[g])
            nc.gpsimd.dma_start(out=wq_sb[g], in_=wq_d[g])
        for g in range(KT // 4):
            nc.gpsimd.dma_start(out=wv_sb[g], in_=wv_d[g])
        for ct in range(HPC):
            nc.gpsimd.dma_start(out=wo_sb[:, ct, :], in_=wo_d[ct])

        # qT/kT per head: [128 rows = (q1 dims 0:64 | q2 dims 64:128), S]
        qT_sb = [singles.tile([128, S], BF16, tag=f"qT{h}", name=f"qT{h}")
                 for h in range(HPC)]
        kT_sb = [singles.tile([128, S], BF16, tag=f"kT{h}", name=f"kT{h}")
                 for h in range(HPC)]
        # v per t-tile: [128 t, 260]: h0 v 0:128, one 128, pad, h1 v 130:258, one 258
        v_sb = singles.tile([128, TT, 260], BF16, tag="v")
        nc.vector.memset(v_sb[:, :, 128:129], 1.0)
        nc.vector.memset(v_sb[:, :, 258:259], 1.0)

        ident = singles.tile([128, 128], BF16, tag="ident")
        make_identity(nc, ident)
        magic = singles.tile([128, 4], I32, tag="magic")
        nc.vector.memset(magic, 0x5F3759DF)
        one_i = singles.tile([128, 1], I32, tag="one_i")
        nc.vector.memset(one_i, 1)

        # ---- Score chunk: one t-tile, both components, one exp ----
        def emit_score_chunk(b, h, t, tiles):
            sblk = slice(b * DBLK, (b + 1) * DBLK)
            tsl = slice(t * 128, (t + 1) * 128)
            sc = psA.tile([128, 2, DBLK], F32, tag="A", name="sc")
            nc.tensor.matmul(sc[:, 0, :], kT_sb[h][0:64, tsl],
                             qT_sb[h][0:64, sblk], start=True, stop=True)
            nc.tensor.matmul(sc[:, 1, :], kT_sb[h][64:128, tsl],
                             qT_sb[h][64:128, sblk], start=True, stop=True)
            e = exp_pool.tile([128, 2, DBLK], BF16, tag="exp")
            nc.scalar.activation(e, sc, Exp)
            tiles[t] = e

        exp_cur = [[None] * TT for _ in range(HPC)]

        # ---- Stage A1: projections (q,k,v in one xt stream) ----
        for b in range(NB):
            sblk = slice(b * DBLK, (b + 1) * DBLK)
            pq = psA.tile([128, 2, DBLK], F32, tag="A", name="pq")
            pk = psA.tile([128, 2, DBLK], F32, tag="A", name="pk")
            pv = [psB.tile([128, 2, CPC], F32, tag="B", name=f"pv{jj}")
                  for jj in range(2)]
            for k in range(KT):
                xt_t = xt_pool.tile([128, DBLK], BF16, tag="xt")
                nc.sync.dma_start(out=xt_t, in_=xt_d[k, b])
                st, sp = (k == 0), (k == KT - 1)
                g, ki = k // 4, k % 4
                for h in range(HPC):
                    nc.tensor.matmul(
                        pq[:, h, :],
                        wq_sb[g][:, ki, h * 128:(h + 1) * 128],
                        xt_t, start=st, stop=sp)
                    nc.tensor.matmul(
                        pk[:, h, :],
                        wk_sb[g][:, ki, h * 128:(h + 1) * 128],
                        xt_t, start=st, stop=sp)
                for j in range(4):
                    nc.tensor.matmul(pv[j // 2][:, j % 2, :],
                                     xt_t[:, j * 128:(j + 1) * 128],
                                     wv_sb[g][:, ki, :],
                                     start=(st and j % 2 == 0), stop=sp)
            for h in range(HPC):
                nc.vector.tensor_copy(qT_sb[h][:, sblk], pq[:, h, :])
                nc.vector.tensor_copy(kT_sb[h][:, sblk], pk[:, h, :])
            for j in range(4):
                t_idx = b * 4 + j
                nc.vector.tensor_copy(v_sb[:, t_idx, 0:128],
                                      pv[j // 2][:, j % 2, 0:128])
                nc.vector.tensor_copy(v_sb[:, t_idx, 130:258],
                                      pv[j // 2][:, j % 2, 128:256])
            for t in BND_SCHED[b]:
                emit_score_chunk(0, 0, t, exp_cur[0])

        # ---- Attention ----
        def emit_gn_j(j, opair, dd_all, rn_all, z_all, mv_all):
            """Per-j GN front half: denominators, z combine, bn stats."""
            nc.vector.tensor_copy(dd_all[:, j, 0:1], opair[:, 0, 128:129])
            rec = gn_pool.tile([128, 1], F32, tag="rec")
            nc.vector.reciprocal(rec, opair[:, 1, 128:129])
            nc.vector.tensor_scalar(
                out=rn_all[:, j:j + 1], in0=rec, scalar1=dd_all[:, j, 0:1],
                scalar2=-lam, op0=Op.mult, op1=Op.mult)
            nc.vector.tensor_copy(z_all[:, j, :], opair[:, 0, 0:128])
            nc.vector.scalar_tensor_tensor(
                out=z_all[:, j, :], in0=opair[:, 1, 0:128],
                scalar=rn_all[:, j:j + 1], in1=z_all[:, j, :],
                op0=Op.mult, op1=Op.add)
            stats = gn_pool.tile([128, 6], F32, tag="stats")
            nc.vector.bn_stats(out=stats, in_=z_all[:, j, :])
            nc.vector.bn_aggr(out=mv_all[:, j, :], in_=stats)

        def emit_gn_finish(h, dd_all, mv_all, z_all, xhs):
            """Batched rsqrt on [128,4] + per-j xh normalize."""
            ww = gn_pool.tile([128, 4], F32, tag="ww")
            nc.vector.tensor_tensor(out=ww, in0=dd_all[:, :, 0],
                                    in1=dd_all[:, :, 0], op=Op.mult)
            nc.vector.tensor_scalar(out=ww, in0=ww, scalar1=EPS,
                                    scalar2=None, op0=Op.mult)
            nc.vector.tensor_tensor(out=ww, in0=mv_all[:, :, 1], in1=ww,
                                    op=Op.add)
            sh = gn_pool.tile([128, 4], I32, tag="sh")
            nc.vector.tensor_scalar(
                out=sh, in0=ww.bitcast(I32), scalar1=one_i,
                scalar2=None, op0=Op.arith_shift_right)
            yy = gn_pool.tile([128, 4], F32, tag="yy")
            nc.vector.tensor_tensor(
                out=yy.bitcast(I32), in0=magic, in1=sh, op=Op.subtract)
            for _ in range(2):
                y2 = gn_pool.tile([128, 4], F32, tag="y2")
                nc.vector.tensor_tensor(out=y2, in0=yy, in1=yy, op=Op.mult)
                nc.vector.tensor_tensor(out=y2, in0=y2, in1=ww, op=Op.mult)
                nc.vector.tensor_scalar(
                    out=y2, in0=y2, scalar1=-0.5, scalar2=1.5,
                    op0=Op.mult, op1=Op.add)
                nyy = gn_pool.tile([128, 4], F32, tag="yy")
                nc.vector.tensor_tensor(out=nyy, in0=yy, in1=y2, op=Op.mult)
                yy = nyy
            for j in range(4):
                xh = gn_pool.tile([128, 128], BF16, tag="xh", bufs=10)
                nc.vector.tensor_scalar(
                    out=xh, in0=z_all[:, j, :], scalar1=mv_all[:, j, 0:1],
                    scalar2=yy[:, j:j + 1], op0=Op.subtract, op1=Op.mult)
                xhs[(h, j)] = xh

        def emit_tr(xh):
            trp = psB.tile([128, 2, CPC], F32, tag="B", name="trp")
            nc.tensor.transpose(trp.bitcast(BF16)[:, 0, 0:128], xh, ident)
            tr = gn_pool.tile([128, 128], BF16, tag="tr", bufs=10)
            nc.vector.tensor_copy(tr, trp.bitcast(BF16)[:, 0, 0:128])
            return tr

        def emit_final(b, j, trs_j):
            srow = (b * 4 + j) * 128
            for n in range(4):
                po = psB.tile([128, 2, CPC], F32, tag="B", name="po")
                dsl = slice(n * DBLK, (n + 1) * DBLK)
                for ct in range(HPC):
                    nc.tensor.matmul(po[:, :, :], trs_j[ct], wo_sb[:, ct, dsl],
                                     start=(ct == 0), stop=(ct == HPC - 1))
                ostage = ost_pool.tile([128, DBLK], BF16, tag="ost")
                nc.vector.tensor_copy(ostage, po[:, :, :])
                nc.sync.dma_start(out=out_d[srow:srow + 128, dsl], in_=ostage)

        phases = [(b, h) for b in range(NB) for h in range(HPC)]
        xhs = {}
        for pi, (b, h) in enumerate(phases):
            nxt = phases[pi + 1] if pi + 1 < len(phases) else None
            exp_tiles = exp_cur[h]
            if nxt is not None:
                exp_cur[nxt[1]] = [None] * TT
            dd_all = gn_pool.tile([128, 4, 2], F32, tag="dd")
            rn_all = gn_pool.tile([128, 4], F32, tag="rn")
            z_all = gn_pool.tile([128, 4, 128], F32, tag="z")
            mv_all = gn_pool.tile([128, 4, 2], F32, tag="mv")
            def emit_pv_half(j, comp, half, opair):
                jsl = slice(j * 128, (j + 1) * 128)
                vsl = slice(h * 130, h * 130 + 129)
                for t in range(half * 8, half * 8 + 8):
                    nc.tensor.matmul(opair[:, comp, 0:129],
                                     exp_tiles[t][:, comp, jsl],
                                     v_sb[:, t, vsl],
                                     start=(comp == 0 and t == 0),
                                     stop=(t == TT - 1))

            for j in range(4):
                opair = psB.tile([128, 2, CPC], F32, tag="B", name="opair")
                for ci, (comp, half) in enumerate(
                        ((0, 0), (0, 1), (1, 0), (1, 1))):
                    if nxt is not None:
                        emit_score_chunk(nxt[0], nxt[1], 4 * j + ci,
                                         exp_cur[nxt[1]])
                    emit_pv_half(j, comp, half, opair)
                emit_gn_j(j, opair, dd_all, rn_all, z_all, mv_all)
            emit_gn_finish(h, dd_all, mv_all, z_all, xhs)
            if h == HPC - 1:
                for j in range(4):
                    trs_j = [emit_tr(xhs[(ct, j)]) for ct in range(HPC)]
                    emit_final(b, j, trs_j)

    nc.compile()
    return nc


def prepare(x, Wq, Wk, Wv, Wo, lambda_q1, lambda_k1, lambda_q2, lambda_k2,
            gn_weight, gn_bias):
    """Host-side sharding/preprocessing. Returns (lam, in_maps, bias_vec)."""
    x = np.asarray(x, dtype=np.float32)
    Wq = np.asarray(Wq, dtype=np.float32)
    Wk = np.asarray(Wk, dtype=np.float32)
    Wv = np.asarray(Wv, dtype=np.float32)
    Wo = np.asarray(Wo, dtype=np.float32)
    gw = np.asarray(gn_weight, dtype=np.float32)
    gb = np.asarray(gn_bias, dtype=np.float32)

    lam = float(np.exp(np.sum(np.asarray(lambda_q1, np.float64)
                              * np.asarray(lambda_k1, np.float64)))
                - np.exp(np.sum(np.asarray(lambda_q2, np.float64)
                                * np.asarray(lambda_k2, np.float64)))
                + LAMBDA_INIT)

    xT = np.ascontiguousarray(
        x.reshape(S, D).T.reshape(KT, 128, NB, DBLK).transpose(0, 2, 1, 3)
    ).astype(NPBF16)
    scale = 1.0 / math.sqrt(DH)

    in_maps = []
    for c in range(NCORES):
        sl = slice(c * CPC, (c + 1) * CPC)
        def _grp(w):
            return np.ascontiguousarray(
                w.reshape(KT // 4, 4, 128, CPC).transpose(0, 2, 1, 3)
                .reshape(KT // 4, 128, 4 * CPC)).astype(NPBF16)
        wq_c = _grp(Wq[:, sl] * scale)
        wk_c = _grp(Wk[:, sl])
        wv_c = _grp(Wv[:, sl])
        wo_c = np.ascontiguousarray(
            Wo[sl, :] * ((1.0 - LAMBDA_INIT) * gw[sl])[:, None]
        ).reshape(HPC, 128, D).astype(NPBF16)
        in_maps.append({"xt": xT, "wq": wq_c, "wk": wk_c, "wv": wv_c,
                        "wo": wo_c})

    bias_vec = ((1.0 - LAMBDA_INIT) * gb.astype(np.float64)) @ Wo.astype(np.float64)
    return lam, in_maps, bias_vec


def kernel(x, Wq, Wk, Wv, Wo, lambda_q1, lambda_k1, lambda_q2, lambda_k2,
           gn_weight, gn_bias):
    lam, in_maps, bias_vec = prepare(
        x, Wq, Wk, Wv, Wo, lambda_q1, lambda_k1, lambda_q2, lambda_k2,
        gn_weight, gn_bias)
    nc = _build(lam)
    res = run_bass_kernel_spmd(nc, in_maps, list(range(NCORES)))
    acc = np.zeros((S, D), dtype=np.float64)
    for c in range(NCORES):
        acc += np.asarray(res.results[c]["out_p"], dtype=np.float64)
    acc += bias_vec[None, :]
    return acc.astype(np.float32).reshape(1, S, D)
